# revision 24
# baseline (speedup 1.0000x reference)
"""Trainium2 Bass kernel for nn_Attention_55233279426826 (block-causal attention).

Reference computation (per batch b):
    xn = LayerNorm(x[b]) * gamma + beta
    q,k,v = split(xn @ w_qkv), 12 heads x 64
    attn  = softmax(block-causal-masked(q k^T / 8))
    out[b] = (attn v) @ w_out + b_out

Sharding (8 cores): batch (2) x head-group (4, 3 heads each).  Each core gets
its batch's x, the w_qkv columns and w_out rows of its 3 heads, and produces a
partial [2048, 768] output.  Host sums the 4 head-group partials per batch and
adds b_out.

v2 design notes (vs the v1 PE-transpose pipeline):
  - xn is transposed via the DMA xbar (dma_start_transpose) into a single
    [128, 6, 2048] tile: dim d = j*128+p.  Frees the PE and two PSUM banks.
  - PE warmup matmuls at t=0 release the HAM clock gate (PE defaults to
    1.2 GHz; ~3.4us of sustained activity unlocks 2.4 GHz).
  - The three heads' scores land in one [128, 3, 512] PSUM tile and are
    exp'ed by a single ACT instruction per J-step (fewer ACT overheads).
  - Block-causal corner masking is done by a tiny zero-weight matmul that
    overwrites the psum corner (start=True), keeping the chain PE-local.
  - AV output is M=128-padded (vaug zero-padded to 128 cols) so FWL hides the
    weight loads; h1's V sits in cols 64:128 so its AV rows land on psum
    partitions 64:128, lining up DVE lanes with the ocat01 packing.
  - Out-projection K-stacks heads 0+1 ([128,T] ocat01) so 3 accumulating
    K=64 matmuls become K=128 + K=64 (2/3 the cycles).
  - QKV / v-layout / finalize / out-projection matmuls are interleaved into
    the attention J-stream as background units so the PE never idles while
    ACT runs exp.
  - LayerNorm apply runs on GpSimd (SBUF-only engine, otherwise idle);
    stats on DVE; exp and half the out-evictions on ACT.
"""

import contextlib
import ctypes
import os
import sys
import types

import ml_dtypes
import numpy as np

B = 2
T = 2048
D = 768
NPATCH = 64
HEADS = 12
DH = 64
NH = 3          # heads per core
CH = 3 * NH * DH  # 576 qkv channels per core
LN_EPS = 1e-5
NCORES = 8

_CACHE = {}


def _install_axon_hooks_shim():
    """This image's antenv lacks axon_hooks; synthesize it so that
    run_bass_kernel_spmd(trace=True) finds the NTFF profile hook instead of
    crashing on import.  Safe no-op if profiling symbols are unavailable."""
    if "antenv.axon_hooks" in sys.modules:
        return
    mod = types.ModuleType("antenv.axon_hooks")
    _hook = [None]
    mod.set_axon_ntff_profile_hook = lambda h: _hook.__setitem__(0, h)
    mod.get_axon_ntff_profile_hook = lambda: _hook[0]
    sys.modules["antenv.axon_hooks"] = mod
    try:
        lib = ctypes.CDLL("/opt/axon/libaxon_pjrt.so")
        if not hasattr(lib, "axon_start_nrt_profile"):
            return
        lib.axon_start_nrt_profile.argtypes = [
            ctypes.POINTER(ctypes.c_int64),
            ctypes.c_size_t,
        ]
        lib.axon_start_nrt_profile.restype = ctypes.c_int64
        lib.axon_stop_nrt_profile.argtypes = [ctypes.c_char_p]
        lib.axon_stop_nrt_profile.restype = ctypes.c_int64

        @contextlib.contextmanager
        def _hook_cm(output_dir, device_ids):
            import jax

            jax.devices()
            if device_ids:
                ids = (ctypes.c_int64 * len(device_ids))(*device_ids)
                rc = lib.axon_start_nrt_profile(ids, len(device_ids))
            else:
                rc = lib.axon_start_nrt_profile(None, 0)
            if rc != 0:
                raise RuntimeError(f"axon_start_nrt_profile rc={rc}")
            try:
                yield
            finally:
                n = lib.axon_stop_nrt_profile(str(output_dir).encode())
                print(f"profile: {n} file(s) -> {output_dir}", file=sys.stderr)

        mod.set_axon_ntff_profile_hook(_hook_cm)
    except OSError:
        pass


def _install_drain_split():
    """The walrus build in this container accepts only ONE sync wait per
    CTRL(drain) instruction; Tile's tail drain carries several.  Split the
    waits across a chain of drains."""
    import bass_rust
    import concourse.tile as tile
    from concourse.vector_clock import ScopedClock

    if getattr(tile.TileContext, "_drain_split_installed", False):
        return

    def _drain_and_barrier(self, tick_clock, wait_clock):
        nc = self.nc
        drain_inst = nc.sync.drain()
        wait_clock.add_sem_waits(
            drain_inst.ins, ScopedClock({None: tick_clock.global_clock})
        )
        si = drain_inst.ins.sync_info
        if si is not None:
            waits = list(si.on_wait)
            if len(waits) > 1:
                si.on_wait = waits[:1]
                for w in waits[1:]:
                    extra = nc.sync.drain()
                    extra.ins.sync_info = bass_rust.SyncInfo(
                        on_wait=[w], on_update=[]
                    )
        nc.all_engine_barrier()
        popped = nc._tile_sem_poison_stack.pop()
        assert popped is self._sem_poison
        nc.clear_and_free_semaphores(list(self.sems.allocated().values()))
        nc.all_engine_barrier()

    tile.TileContext._drain_and_barrier = _drain_and_barrier

    # Generic pass: walrus here allows 1 sync wait per instruction; move
    # extra waits onto nofuse NOPs inserted just before, on the same engine.
    from concourse import mybir

    orig_lower = tile.TileContext._lower_ordered_insts

    def _lower_split(self, ordered):
        for insts in ordered.values():
            new = []
            for inst in insts:
                si = getattr(inst, "sync_info", None)
                eng = getattr(inst, "engine", None)
                if si is not None and eng is not None:
                    waits = list(si.on_wait)
                    if len(waits) > 1:
                        movable = [w for w in waits
                                   if getattr(w, "sync_type", "") == "semaphore"]
                        keep = [w for w in waits if w not in movable]
                        if not keep:
                            keep = [movable.pop()]
                        for k, w in enumerate(movable):
                            nop = mybir.InstNoOp(
                                name=f"{inst.name}-wsplit{k}",
                                sync_info=mybir.SyncInfo(
                                    on_wait=[w], on_update=[]
                                ),
                                bass_nofuse=True,
                                engine=eng,
                            )
                            new.append(nop)
                        inst.sync_info = mybir.SyncInfo(
                            on_wait=keep, on_update=list(si.on_update)
                        )
                new.append(inst)
            insts[:] = new
        return orig_lower(self, ordered)

    tile.TileContext._lower_ordered_insts = _lower_split
    tile.TileContext._drain_split_installed = True


# qkvT row layout: which [128/64, 2048] tile and partition offset holds each
# head's 64-row qT/kT/vT strip.  q and k of the same head share a partition
# offset (matmul operands must have equal base partitions).
Q_LOC = [(0, 0), (0, 64), (2, 0)]
K_LOC = [(1, 0), (1, 64), (3, 0)]
V_LOC = [(2, 64), (3, 64), (4, 0)]
# host column order of the permuted per-core w_qkv (64-col segments)
# tile0 = [q0; q1], tile1 = [k0; k1], tile2 = [q2; v0], tile3 = [k2; v1],
# tile4 = [v2]
SEG_ORDER = [("q", 0), ("q", 1), ("k", 0), ("k", 1), ("q", 2), ("v", 0),
             ("k", 2), ("v", 1), ("v", 2)]

C_CHUNKS = [(0, 128), (128, 128), (256, 128), (384, 128), (512, 64)]

# vaug column packing per head: (data col base, den col).  h1's data goes in
# cols 64:128 so its AV psum rows land on partitions 64:128 (DVE lanes then
# line up with the upper half of ocat01); its denominator row is row 0.
V_DATA = [0, 64, 0]
V_DEN = [64, 0, 64]
# ocat destination: (tile_idx 0->ocat01 / 1->ocat2, partition base)
O_DST = [(0, 0), (0, 64), (1, 0)]


def build_nc():
    import concourse.bass as bass
    import concourse.tile as tile
    from concourse import mybir
    from concourse.masks import make_identity

    _install_drain_split()

    f32 = mybir.dt.float32
    f32r = mybir.dt.float32r
    bf16 = mybir.dt.bfloat16
    AF = mybir.ActivationFunctionType
    Alu = mybir.AluOpType

    nc = bass.Bass()
    x_d = nc.dram_tensor("x", [T, D], f32, kind="ExternalInput")
    # gamma is folded into w_qkv on the host; bw = beta @ w_qkv_orig is
    # precomputed on the host in [5, 128] chunk-major layout.
    wqkv_d = nc.dram_tensor("wqkv", [D, CH], bf16, kind="ExternalInput")
    wout_d = nc.dram_tensor("wout", [NH * DH, D], bf16, kind="ExternalInput")
    bw_d = nc.dram_tensor("bw", [5, 128], f32, kind="ExternalInput")
    out_d = nc.dram_tensor("out", [T, D], f32, kind="ExternalOutput")

    with contextlib.ExitStack() as ctx:
        ctx.enter_context(
            nc.allow_low_precision(reason="bf16 PE inputs are intentional")
        )
        tc = ctx.enter_context(tile.TileContext(nc))
        consts = ctx.enter_context(tc.tile_pool(name="consts", bufs=1))
        wpool = ctx.enter_context(tc.tile_pool(name="w", bufs=1))
        xnT_pool = ctx.enter_context(tc.tile_pool(name="xnTp", bufs=1))
        qkvT_pool = ctx.enter_context(tc.tile_pool(name="qkvT", bufs=1))
        vaug_pool = ctx.enter_context(tc.tile_pool(name="vaug", bufs=1))
        ocat_pool = ctx.enter_context(tc.tile_pool(name="ocat", bufs=1))
        pt_pool = ctx.enter_context(tc.tile_pool(name="pt", bufs=2))  # per-tag double buffer
        io_pool = ctx.enter_context(tc.tile_pool(name="io", bufs=4))
        osb_pool = ctx.enter_context(tc.tile_pool(name="osb", bufs=3))
        stats = ctx.enter_context(tc.tile_pool(name="stats", bufs=4))
        rec_pool = ctx.enter_context(tc.tile_pool(name="rec", bufs=2))
        xn_pool = ctx.enter_context(tc.tile_pool(name="xn", bufs=16))
        # PSUM: shared rotation (2 banks) + score stall (3) + AV accum (3)
        shp = ctx.enter_context(tc.tile_pool(name="shp", bufs=2, space="PSUM"))
        st01_ps = ctx.enter_context(tc.tile_pool(name="st01", bufs=1, space="PSUM"))
        st2_ps = ctx.enter_context(tc.tile_pool(name="st2", bufs=1, space="PSUM"))
        ot_ps = ctx.enter_context(tc.tile_pool(name="ot", bufs=1, space="PSUM"))

        def shp_tile():
            t = shp.tile([128, 512], f32, tag="shp", name="shp")
            return t

        # ---- PE warmup: the HAM clock gate defaults to 1.2 GHz; ~3.5us of
        # back-to-back matmuls unlocks 2.4 GHz before real work arrives.
        wscr = consts.tile([128, 512], bf16, tag="wscr")
        nc.vector.memset(wscr, 0.0)
        for _ in range(14):
            wt = shp_tile()
            nc.tensor.matmul(wt, wscr[:, 0:128], wscr, start=True, stop=True)

        identity = consts.tile([128, 128], f32, tag="id")
        make_identity(nc, identity)
        id_bf = consts.tile([128, 128], bf16, tag="idbf")
        nc.vector.tensor_copy(id_bf, identity)
        negbig = consts.tile([64, 64], bf16, tag="nb64")
        nc.vector.memset(negbig, -200.0)
        eps_t = consts.tile([128, 1], f32, tag="eps")
        nc.vector.memset(eps_t, LN_EPS)
        ones128 = consts.tile([1, 128], f32r, tag="ones")
        nc.vector.memset(ones128.bitcast(f32), 1.0)

        # ---- xnT: [128, 6, 512] bf16 per 512-token group, filled by DMA
        # xbar transposes.  dim d = j*128 + p (verified on hw), token axis
        # unpermuted.  Per-group tiles keep QKV(g) deps off other groups' LN.
        xnT = [xnT_pool.tile([128, 6, 512], bf16, tag=f"xnT{g}", name=f"xnT{g}")
               for g in range(4)]

        # ---- DMA issue order: group-0 x tiles, then weights, then the rest
        # of x.  All plain loads issue before any dma_start_transpose so a
        # transpose waiting on its xn can't head-of-line-block the SP queue.
        xts = []
        for i in range(4):
            xt = io_pool.tile([128, D], f32, tag="xin", name="xin")
            nc.sync.dma_start(xt, x_d[128 * i : 128 * (i + 1), :])
            xts.append(xt)
        w_sb = []
        for j in range(6):
            wf = wpool.tile([128, CH], bf16, tag=f"w{j}", name=f"w{j}")
            nc.sync.dma_start(wf, wqkv_d[128 * j : 128 * (j + 1), :])
            w_sb.append(wf)
        wout01 = wpool.tile([128, D], bf16, tag="wo01", name="wo01")
        nc.sync.dma_start(wout01, wout_d[0:128, :])
        wout2 = wpool.tile([64, D], bf16, tag="wo2", name="wo2")
        nc.sync.dma_start(wout2, wout_d[128:192, :])
        wout_sb = [wout01, wout2]
        bw_raw = wpool.tile([5, 128], f32, tag="bwr", name="bwr")
        nc.sync.dma_start(bw_raw, bw_d[:, :])
        # x4-15 in two batched loads on the gpsimd SWDGE queue so neither
        # the scalar input queue nor the sync transpose queue carries them.
        xmid = io_pool.tile([128, 4, D], f32, tag="xmid", name="xmid", bufs=1)
        nc.gpsimd.dma_start(
            xmid, x_d[512:1024, :].rearrange("(a p) d -> p a d", p=128)
        )
        xbig = io_pool.tile([128, 8, D], f32, tag="xbig", name="xbig", bufs=1)
        nc.gpsimd.dma_start(
            xbig, x_d[1024:T, :].rearrange("(a p) d -> p a d", p=128)
        )

        def ln_unit(i):
            if i < 4:
                xt = xts[i]
            elif i < 8:
                xt = xmid[:, i - 4, :]
            else:
                xt = xbig[:, i - 8, :]
            bnst = stats.tile([128, 3, 6], f32, tag="bnst", name="bnst")
            for s in range(3):
                nc.vector.bn_stats(bnst[:, s, :], xt[:, 256 * s : 256 * (s + 1)])
            mv = stats.tile([128, 2], f32, tag="mv", name="mv")
            nc.vector.bn_aggr(mv, bnst)
            rstd = stats.tile([128, 1], f32, tag="rstd", name="rstd")
            nc.scalar.activation(rstd, mv[:, 1:2], AF.Sqrt, bias=eps_t)
            nc.vector.reciprocal(rstd, rstd)
            xn_t = xn_pool.tile([128, D], bf16, tag="xnt", name="xnt")
            if i < 8:
                # ACT is idle during the prefix; DVE is the serial bottleneck
                nmr = stats.tile([128, 1], f32, tag="nmr", name="nmr")
                nc.vector.tensor_scalar(
                    out=nmr, in0=mv[:, 0:1], scalar1=rstd, scalar2=-1.0,
                    op0=Alu.mult, op1=Alu.mult,
                )
                nc.scalar.activation(
                    xn_t, xt, AF.Identity, bias=nmr, scale=rstd
                )
            else:
                nc.vector.tensor_scalar(
                    out=xn_t,
                    in0=xt,
                    scalar1=mv[:, 0:1],
                    scalar2=rstd,
                    op0=Alu.subtract,
                    op1=Alu.mult,
                )
            nc.sync.dma_start_transpose(
                xnT[i // 4][:, 0:6, 128 * (i % 4) : 128 * (i % 4 + 1)], xn_t
            )

        for i in range(4):
            ln_unit(i)

        # ---- bw [5,128] -> [128,5] via one PE transpose
        bwp = shp_tile()
        nc.tensor.matmul(
            bwp[:, 0:5], bw_raw, identity[0:5, 0:5],
            start=True, stop=True, is_transpose=True,
        )
        bwT = wpool.tile([128, 5], f32, tag="bwT", name="bwT")
        nc.vector.tensor_copy(bwT, bwp[:, 0:5])


        qkvT = []
        for ci, (clo, csz) in enumerate(C_CHUNKS):
            qkvT.append(
                qkvT_pool.tile([csz, T], bf16, tag=f"qkvT{ci}", name=f"qkvT{ci}")
            )
        vaug = [
            vaug_pool.tile([128, 16, 128], bf16, tag=f"va{h}", name=f"va{h}")
            for h in range(NH)
        ]
        for h in range(NH):
            nc.vector.memset(vaug[h][:, :, :], 0.0)
            nc.vector.memset(
                vaug[h][:, :, V_DEN[h] : V_DEN[h] + 1].bitcast(bf16), 1.0
            )
        ocat01 = ocat_pool.tile([128, T], bf16, tag="oc01", name="oc01")
        ocat2 = ocat_pool.tile([64, T], bf16, tag="oc2", name="oc2")
        ocat_t = [ocat01, ocat2]

        # ---------- work units ----------
        def q_unit(g, ci):
            clo, csz = C_CHUNKS[ci]
            pq = shp_tile()
            for j in range(6):
                nc.tensor.matmul(
                    pq[:csz, :],
                    w_sb[j][:, clo : clo + csz],
                    xnT[g][:, j, :],
                    start=(j == 0),
                    stop=(j == 5),
                )
            nc.vector.tensor_scalar_add(
                qkvT[ci][:csz, 512 * g : 512 * (g + 1)],
                in0=pq[:csz, :],
                scalar1=bwT[:csz, ci : ci + 1],
            )

        def vtr_unit(g, h):
            tI, ro = V_LOC[h]
            idsl = id_bf[ro : ro + 64, ro : ro + 64]
            vt = shp_tile().bitcast(bf16)
            for u in range(4):
                J = 4 * g + u
                nc.tensor.transpose(
                    vt[:, 64 * u : 64 * (u + 1)],
                    qkvT[tI][ro : ro + 64, 128 * J : 128 * (J + 1)],
                    idsl,
                )
            dc = V_DATA[h]
            nc.vector.tensor_copy(
                vaug[h][:, 4 * g : 4 * (g + 1), dc : dc + 64],
                vt[:, 0:256].rearrange("p (u d) -> p u d", u=4),
            )

        def fin_unit(c, h, otp):
            dr = V_DEN[h]          # den row in otp[h]
            db = V_DATA[h]         # data partition base in otp[h]
            oti, ob = O_DST[h]
            # 1/den via exp(-ln(den)) on ACT (table ops are the cheapest
            # per-element reciprocal on this build), broadcast via K=1 matmul
            ld = rec_pool.tile([1, 512], f32, tag="ld", name="ld")
            nc.scalar.activation(ld, otp[h][dr : dr + 1, :], AF.Ln)
            rec = rec_pool.tile([1, 512], f32r, tag="rec", name="rec")
            nc.scalar.activation(rec, ld, AF.Exp, scale=-1.0)
            bc = shp_tile()
            nc.tensor.matmul(bc, ones128, rec, start=True, stop=True)
            recs = rec_pool.tile([128, 512], f32, tag="recs", name="recs")
            nc.vector.tensor_copy(recs[ob : ob + 64, :], bc[ob : ob + 64, :])
            nc.vector.tensor_mul(
                ocat_t[oti][ob : ob + 64, 512 * c : 512 * (c + 1)],
                otp[h][db : db + 64, :],
                recs[ob : ob + 64, :],
            )

        def op_unit(c, t):
            osb = osb_pool.tile([128, D], f32, tag="osb", name="osb")
            for eh in range(2):
                op = shp_tile()
                nc.tensor.matmul(
                    op[:, 0:384],
                    ocat01[:, 128 * t : 128 * (t + 1)],
                    wout01[:, 384 * eh : 384 * (eh + 1)],
                    start=True,
                    stop=False,
                )
                nc.tensor.matmul(
                    op[:, 0:384],
                    ocat2[:, 128 * t : 128 * (t + 1)],
                    wout2[:, 384 * eh : 384 * (eh + 1)],
                    start=False,
                    stop=True,
                )
                nc.vector.tensor_copy(
                    osb[:, 384 * eh : 384 * (eh + 1)], op[:, 0:384]
                )
            nc.sync.dma_start(out_d[128 * t : 128 * (t + 1), :], osb)

        # ---- QKV + v-layout for group 0 up front (before LN 4-7 so the
        # first scores don't queue behind LN work on DVE); later groups, the
        # finalize, and the out-projection interleave into the J-stream.
        for ci in range(5):
            q_unit(0, ci)
        for h in range(NH):
            vtr_unit(0, h)
        for i in range(4, 8):
            ln_unit(i)

        def warm_unit():
            wt = shp_tile()
            nc.tensor.matmul(wt, wscr[:, 0:128], wscr, start=True, stop=True)

        # fillers keep the PE (and its HAM clock gate) busy while QKV(1)
        # waits on the HBM-bound LN/transpose chain for group 1.
        bg = [warm_unit for _ in range(12)]
        bg += [lambda ci=ci: q_unit(1, ci) for ci in range(5)]
        bg += [lambda h=h: vtr_unit(1, h) for h in range(NH)]
        bg += [lambda i=i: ln_unit(i) for i in range(8, 12)]

        def pump(k):
            for _ in range(k):
                if bg:
                    bg.pop(0)()

        scale = float(DH) ** -0.5
        for c in range(4):
            nJ = 4 * c + 4
            rate = max(2, -(-len(bg) // nJ))
            otp = [
                ot_ps.tile([128, 512], f32, tag=f"ot{h}", name=f"ot{h}")
                for h in range(NH)
            ]
            def score_mm(out_ap, h, J, q0, n):
                qt, qo = Q_LOC[h]
                kt, ko = K_LOC[h]
                nc.tensor.matmul(
                    out_ap,
                    qkvT[kt][ko : ko + 64, 128 * J : 128 * (J + 1)],
                    qkvT[qt][qo : qo + 64, q0 : q0 + n],
                    start=True,
                    stop=True,
                )

            def emit_av(Jp, s0p, np_, pt01p, pt2p):
                for h in range(NH):
                    rhs = pt2p[:, 0:np_] if h == 2 else pt01p[:, h, 0:np_]
                    nc.tensor.matmul(
                        otp[h][:, s0p:512],
                        vaug[h][:, Jp, :],
                        rhs,
                        start=(Jp == 0),
                        stop=(Jp == nJ - 1),
                    )

            pending = None
            for J in range(nJ):
                s0 = max(0, 128 * J - 512 * c)
                n = 512 - s0
                q0 = 512 * c + s0
                diag = J >= 4 * c
                # h2 first: its exp drains while h0/h1 scores stream, so the
                # next step's h2 score never waits on a just-issued exp.
                stt2 = st2_ps.tile([128, 512], f32, tag="st2", name="st2")
                score_mm(stt2[:, 0:n], 2, J, q0, n)
                if diag:
                    nc.tensor.matmul(
                        stt2[64:128, 0:64], id_bf[0:64, 0:64], negbig,
                        start=True, stop=True,
                    )
                pt2 = pt_pool.tile([128, 512], bf16, tag="pt2", name="pt2")
                nc.scalar.activation(pt2[:, 0:n], stt2[:, 0:n], AF.Exp, scale=scale)
                stt01 = st01_ps.tile([128, 2, 512], f32, tag="st01", name="st01")
                for h in range(2):
                    score_mm(stt01[:, h, 0:n], h, J, q0, n)
                    if diag:
                        nc.tensor.matmul(
                            stt01[64:128, h, 0:64], id_bf[0:64, 0:64], negbig,
                            start=True, stop=True,
                        )
                pt01 = pt_pool.tile([128, 2, 512], bf16, tag="pt01", name="pt01")
                nc.scalar.activation(
                    pt01[:, :, 0:n], stt01[:, :, 0:n], AF.Exp, scale=scale
                )
                pump(rate)
                if pending is not None:
                    emit_av(*pending)
                pending = (J, s0, n, pt01, pt2)
            emit_av(*pending)
            # refill background work: finalize+project this chunk (pumped
            # during the next chunk), then QKV/v for group c+2.
            bg += [lambda h=h, c=c, otp=otp: fin_unit(c, h, otp)
                   for h in range(NH)]
            if c + 2 <= 3:
                bg += [lambda ci=ci, g=c + 2: q_unit(g, ci) for ci in range(5)]
                bg += [lambda h=h, g=c + 2: vtr_unit(g, h) for h in range(NH)]
            bg += [lambda c=c, t=t: op_unit(c, t)
                   for t in range(4 * c, 4 * c + 4)]
            if c == 0:
                bg += [lambda i=i: ln_unit(i) for i in range(12, 16)]
        while bg:
            bg.pop(0)()

    return nc


def shard_inputs(x, gamma, beta, w_qkv, w_out, b_out):
    """Full inputs -> list of 8 per-core input dicts."""
    x = np.ascontiguousarray(np.asarray(x, dtype=np.float32))
    gamma = np.asarray(gamma, dtype=np.float32)
    beta = np.asarray(beta, dtype=np.float32)
    w_qkv = np.asarray(w_qkv, dtype=np.float32)
    w_out = np.asarray(w_out, dtype=np.float32)
    in_maps = []
    for g in range(NCORES):
        b = g // 4
        hg = g % 4
        heads = [3 * hg + h for h in range(NH)]
        segs = []
        for kind, h in SEG_ORDER:
            hh = heads[h]
            base = {"q": 0, "k": D, "v": 2 * D}[kind]
            segs.append(w_qkv[:, base + 64 * hh : base + 64 * (hh + 1)])
        wqkv_g = np.ascontiguousarray(np.concatenate(segs, axis=1))
        wout_g = np.ascontiguousarray(
            w_out[64 * heads[0] : 64 * (heads[-1] + 1), :]
        )
        bw_g = beta @ wqkv_g  # [576]
        bw_raw = np.zeros((5, 128), dtype=np.float32)
        for ci, (clo, csz) in enumerate(C_CHUNKS):
            bw_raw[ci, :csz] = bw_g[clo : clo + csz]
        in_maps.append(
            {
                "x": x[b],
                "wqkv": (gamma[:, None] * wqkv_g).astype(ml_dtypes.bfloat16),
                "wout": wout_g.astype(ml_dtypes.bfloat16),
                "bw": bw_raw,
            }
        )
    return in_maps


def kernel(x, gamma, beta, w_qkv, w_out, b_out):
    _install_axon_hooks_shim()
    from concourse import bass_utils

    if "nc" not in _CACHE:
        _CACHE["nc"] = build_nc()
    nc = _CACHE["nc"]

    in_maps = shard_inputs(x, gamma, beta, w_qkv, w_out, b_out)
    trace = bool(int(os.environ.get("KERNEL_TRACE", "0")))
    kwargs = {}
    if trace:
        kwargs["trace"] = True
        tmpdir = os.environ.get("KERNEL_TRACE_DIR")
        if tmpdir:
            kwargs["tmpdir"] = tmpdir
        # artifact upload needs external storage; keep the trace local
        bass_utils.upload_artifacts = lambda d: d
    res = bass_utils.run_bass_kernel_spmd(
        nc, in_maps, list(range(NCORES)), **kwargs
    )
    _CACHE["last_exec_time_ns"] = res.exec_time_ns

    b_out = np.asarray(b_out, dtype=np.float32)
    out = np.empty((B, T, D), dtype=np.float32)
    for b in range(B):
        acc = res.results[4 * b]["out"].astype(np.float32)
        for hg in range(1, 4):
            acc = acc + res.results[4 * b + hg]["out"]
        out[b] = acc + b_out[None, :]
    return out
